# revision 1
# baseline (speedup 1.0000x reference)
"""GATv2 2-layer GNN + global mean pool on 8 TRN2 NeuronCores (Bass/Tile).

v3 on top of v2: the attention dot product is algebraically refactored so the
per-edge DVE work is one add + one abs + a 42-wide signed tree:
  att.leaky(s) = 0.6*att.s + 0.4*att.|s|   (leaky slope 0.2)
Host premultiplies BOTH tables by 0.4*att (columns permuted per head:
positive-att columns in [84h,84h+42), negative in [84h+42,84h+84), zero pad;
3x84=252 cols + ones col (252) + 3 "al/ar" columns (253:256) holding the
precomputed 0.6*att.xl / 0.6*att.xr linear parts, all folded into the layer
weight matrices). Per edge: u = xl'[src]+xr'[dst] (one DVE add), |u| via 4x
tensor_scalar abs_max, logit = (sum_A |u| - sum_B |u|) + (al+ar), exp.
Aggregation rhs = raw premultiplied xl' (ones col 252 accumulates the
denominator); slot-major normalization via 4 per-bank per-partition
tensor_scalar multiplies. Recovery of true features (divide by 0.4*att -
exact, a pure product) happens in the node-major pass: layer 1 multiplies by
the winv vector then +b, relu (layer-2 weights are row-permuted to match);
layer 2 folds recovery + column unpermute + head-mean into a single [252,64]
Pmat applied by PE after an on-chip transpose. xr is never gathered per edge:
a per-chunk TensorE matmul broadcasts the bank's 40 xr' rows via streamed
one-hot selT; PSUM pairs of chunks are copied to SBUF by ACT.
"""
import sys

sys.path.insert(0, "/opt/trn_rl_repo")

import numpy as np
import ml_dtypes

import concourse.bass as bass
import concourse.mybir as mybir
import concourse.tile as tile
import concourse.bacc as bacc
from concourse import bass_utils
from concourse.masks import make_identity

BF16 = mybir.dt.bfloat16
F32 = mybir.dt.float32
I16 = mybir.dt.int16

N, E, F, H, C, G, NCLS = 50000, 800000, 128, 3, 64, 16, 10
NCORES = 8
BANK_NODES = 40
NBANKS = 4
SET_NODES = BANK_NODES * NBANKS   # 160
NSETS = 40
NPAD = NSETS * SET_NODES          # 6400 padded node positions per core
NG = NCORES * NPAD                # 51200 global table rows
HALFG = NG // 2                   # 25600
DPAD = 256                        # table row (bf16) -> 512B
BW = 84                           # per-head block: 42 pos | 42 neg (padded)
FW = H * BW                       # 252 feature cols
DW = FW + 1                       # 253: + ones/den col
MTILES = NPAD // 128              # 50
HROW = FW                         # node-major row width


# ------------------------------------------------------------------
# host preprocessing
# ------------------------------------------------------------------

def _wrap16(seq):
    n = seq.size
    w = np.asarray(seq, np.int16).reshape(n // 16, 16).T
    return np.ascontiguousarray(np.tile(w, (8, 1)))


def _permute_nodes(src_g, dst_g):
    """node -> (core, pos). Degree-sorted deal; within each rank-8 group a
    greedy assignment balances per-(core,set,bank) even-parity degree sums
    (gather halves are even/odd table rows; a node's slot parity is
    group-invariant so the balance is not circular)."""
    deg = np.bincount(dst_g, minlength=N)
    order = np.argsort(-deg, kind="stable")
    r = np.arange(N)
    q = r // NCORES
    binid = q % SET_NODES
    st = binid % NSETS
    bk = binid // NSETS
    sl = q // SET_NODES
    pos_r = st * SET_NODES + bk * BANK_NODES + sl
    pos_of = np.empty(N, np.int64)
    pos_of[order] = pos_r
    evdeg = np.bincount(dst_g[pos_of[src_g] % 2 == 0], minlength=N)
    core_of = np.empty(N, np.int64)
    S = np.zeros((NCORES, NSETS, NBANKS))
    for t in range(N // NCORES):
        g = order[8 * t:8 * t + 8]
        b_, s_ = bk[8 * t], st[8 * t]
        nds = sorted(g, key=lambda n: -evdeg[n])
        cs = sorted(range(NCORES), key=lambda c: S[c, s_, b_])
        for n, c in zip(nds, cs):
            core_of[n] = c
            S[c, s_, b_] += evdeg[n]
    return core_of, pos_of


def _preprocess(edge_index, batch):
    src_g = np.concatenate([np.asarray(edge_index[0], np.int64),
                            np.arange(N, dtype=np.int64)])
    dst_g = np.concatenate([np.asarray(edge_index[1], np.int64),
                            np.arange(N, dtype=np.int64)])
    core_of, pos_of = _permute_nodes(src_g, dst_g)
    gid = core_of * NPAD + pos_of                     # permuted global row id

    per_core = []
    counts = np.zeros((NCORES, NSETS, NBANKS, 2), np.int64)
    for c in range(NCORES):
        m = core_of[dst_g] == c
        src = gid[src_g[m]]                           # permuted global src row
        dpos = pos_of[dst_g[m]]
        half = src % 2                                # gather half = row parity
        order = np.argsort(dpos * 2 + half, kind="stable")
        src, dpos, half = src[order], dpos[order], half[order]
        bank_id = dpos // BANK_NODES
        set_id = bank_id // NBANKS
        bank = bank_id % NBANKS
        np.add.at(counts[c], (set_id, bank, half), 1)
        per_core.append((src, dpos, set_id, bank, half))

    # round bank capacities to 32 and nudge so every intra-chunk segment
    # boundary is a legal PE base partition (0/32/64 -- 96 is not allowed)
    cap = -(-counts.max(axis=0) // 32) * 32           # [NSETS, NBANKS, 2]
    for k in range(NSETS):
        for hf in range(2):
            cum = 0
            for b in range(NBANKS):
                if (cum + cap[k, b, hf]) % 128 == 96 and b < NBANKS - 1:
                    cap[k, b, hf] += 32
                cum += cap[k, b, hf]
    reg_ch = -(-cap.sum(axis=1) // 128)               # chunks per (set, half)
    tot_ch = int(reg_ch.sum())
    tot_slots = tot_ch * 128

    # straddle-packed region layout: banks back-to-back, jobs carry intra-
    # chunk slot ranges; per-chunk xr segments cover [0,128) (pads have
    # all-zero one-hots so over-coverage is harmless)
    jobs = [[] for _ in range(NSETS)]       # (col, b, lo, hi, start, stop)
    chunk_segs = [[] for _ in range(NSETS)] # per chunk: [(b, lo, hi)]
    seg_base = np.zeros((NSETS, 2, NBANKS), np.int64)  # slot offset of bank seg
    reg_base = np.zeros((NSETS, 2), np.int64)          # global slot of region
    set_nch = []
    reg_off = []
    pos_slots = 0
    for k in range(NSETS):
        col = 0
        offs = []
        segs_all = []
        for hf in range(2):
            a = pos_slots
            reg_base[k, hf] = pos_slots
            nch_hf = int(reg_ch[k, hf])
            off = 0
            bsegs = []
            for b in range(NBANKS):
                seg_base[k, hf, b] = off
                bsegs.append((b, off, off + int(cap[k, b, hf])))
                off += int(cap[k, b, hf])
            assert off <= nch_hf * 128
            # extend last segment to region end (pad slots select nothing)
            b, s0, _ = bsegs[-1]
            bsegs[-1] = (b, s0, nch_hf * 128)
            def _legal_pieces(l, h):
                # PE tile positions: base 0 (<=128 rows), 32 (<=32), 64 (<=64)
                out = []
                while l < h:
                    if l == 0:
                        out.append((0, h)); l = h
                    elif l == 32:
                        out.append((32, min(h, 64))); l = min(h, 64)
                    elif l == 64:
                        out.append((64, h)); l = h
                    else:
                        raise AssertionError(f"illegal segment base {l}")
                return out

            for cc in range(nch_hf):
                lo_c, hi_c = cc * 128, (cc + 1) * 128
                segs = []
                for (b, s0, s1) in bsegs:
                    l, h = max(s0, lo_c), min(s1, hi_c)
                    if l < h:
                        for (pl, ph) in _legal_pieces(l - lo_c, h - lo_c):
                            segs.append((b, pl, ph))
                chunk_segs[k].append(segs)
                for (b, l, h) in segs:
                    jobs[k].append([col, b, l, h, False, False])
                col += 1
            pos_slots += nch_hf * 128
            offs.append((a, pos_slots))
        # start/stop flags per bank across the whole set
        firstj = {}
        lastj = {}
        for j, jb in enumerate(jobs[k]):
            b = jb[1]
            if b not in firstj:
                firstj[b] = j
            lastj[b] = j
        for b in firstj:
            jobs[k][firstj[b]][4] = True
            jobs[k][lastj[b]][5] = True
        set_nch.append(col)
        reg_off.append(offs)
    assert sum(set_nch) == tot_ch
    assert pos_slots == tot_slots

    cores = []
    for c in range(NCORES):
        src, dpos, set_id, bank, half = per_core[c]
        xl_idx = np.zeros(tot_slots, np.int64)
        selpat = np.full(tot_slots, -1, np.int64)
        for k in range(NSETS):
            for hf in range(2):
                for b in range(NBANKS):
                    selm = (set_id == k) & (bank == b) & (half == hf)
                    n = int(selm.sum())
                    p0 = int(reg_base[k, hf] + seg_base[k, hf, b])
                    assert n <= cap[k, b, hf]
                    xl_idx[p0:p0 + n] = src[selm] // 2
                    selpat[p0:p0 + n] = dpos[selm] % BANK_NODES
        # s-major aggregation one-hots: col 3*node_in_bank + head
        sel = np.zeros((128, tot_ch, 120), ml_dtypes.bfloat16)
        sp = selpat.reshape(tot_ch, 128)
        ii, jj = (sp >= 0).nonzero()     # ii = chunk, jj = row
        for hh in range(H):
            sel[jj, ii, H * sp[ii, jj] + hh] = 1.0
        # xr-broadcast one-hots: selT[node_in_bank, slot]
        selT = np.zeros((BANK_NODES, tot_slots), ml_dtypes.bfloat16)
        real = selpat >= 0
        selT[selpat[real], np.nonzero(real)[0]] = 1.0
        cores.append(dict(
            xl_idx16=_wrap16(xl_idx),
            sel01=np.ascontiguousarray(sel.reshape(128, tot_ch * 120)),
            selT=np.ascontiguousarray(selT)))

    meta = dict(jobs=jobs, set_nch=set_nch, reg_off=reg_off,
                chunk_segs=chunk_segs, tot_ch=tot_ch, tot_slots=tot_slots,
                core_of=core_of, pos_of=pos_of)
    return cores, meta


def _onehots(batch, core, core_of, pos_of):
    oh = np.zeros((128, MTILES, G), ml_dtypes.bfloat16)
    bat = np.asarray(batch, np.int64)
    mine = np.nonzero(core_of == core)[0]
    p = pos_of[mine]
    oh[p % 128, p // 128, bat[mine]] = 1.0
    return np.ascontiguousarray(oh.reshape(128, MTILES * G))


# ------------------------------------------------------------------
# device builder
# ------------------------------------------------------------------

def _build(meta, reps=1):
    nc = bacc.Bacc(num_swdge_queues=3)
    jobs, set_nch, reg_off = meta["jobs"], meta["set_nch"], meta["reg_off"]
    chunk_segs = meta["chunk_segs"]
    tot_ch, tot_slots = meta["tot_ch"], meta["tot_slots"]

    xT = nc.declare_dram_parameter("xT", [F, NPAD], BF16, isOutput=False)
    wl1 = nc.declare_dram_parameter("wl1", [F, DPAD], BF16, isOutput=False)
    wr1 = nc.declare_dram_parameter("wr1", [F, DPAD], BF16, isOutput=False)
    wl2 = nc.declare_dram_parameter("wl2", [FW, DPAD], BF16, isOutput=False)
    wr2 = nc.declare_dram_parameter("wr2", [FW, DPAD], BF16, isOutput=False)
    w1inv_in = nc.declare_dram_parameter("w1inv_rep", [128, FW], BF16, isOutput=False)
    b1p_in = nc.declare_dram_parameter("b1p_rep", [128, FW], BF16, isOutput=False)
    pmat_in = nc.declare_dram_parameter("pmat", [FW, C], BF16, isOutput=False)
    b2_rep = nc.declare_dram_parameter("b2_rep", [128, 64], F32, isOutput=False)
    wc_in = nc.declare_dram_parameter("wc", [C, NCLS], F32, isOutput=False)
    bc_rep = nc.declare_dram_parameter("bc_rep", [G, NCLS], F32, isOutput=False)
    cntr = nc.declare_dram_parameter("cnt_recip", [G, 1], F32, isOutput=False)
    oh_in = nc.declare_dram_parameter("oh", [128, MTILES * G], BF16, isOutput=False)
    xl_idx = nc.declare_dram_parameter("xl_idx16", [128, tot_slots // 16], I16, isOutput=False)
    sel_in = nc.declare_dram_parameter("sel01", [128, tot_ch * 120], BF16, isOutput=False)
    selT_in = nc.declare_dram_parameter("selT", [BANK_NODES, tot_slots], BF16, isOutput=False)
    out_ext = nc.declare_dram_parameter("out", [G, NCLS], F32, isOutput=True)

    import os
    _shared = os.environ.get("SHARED_GLOB", "0") == "1"
    shard_tab = nc.dram_tensor("shard_tab", [NPAD, DPAD], BF16)
    glob_tab = nc.dram_tensor("glob_tab", [NG, DPAD], BF16,
                              addr_space="Shared" if _shared else "Local")
    xr_tab = nc.dram_tensor("xr_tab", [NPAD, DPAD], BF16)
    h_slots = nc.dram_tensor("h_slots", [NSETS, 128, NBANKS * FW], BF16)
    h1_node = nc.dram_tensor("h1_node", [NPAD, HROW], BF16)
    o2_node = nc.dram_tensor("o2_node", [NPAD, HROW], BF16)
    pool_in = nc.dram_tensor("pool_in", [G, C], F32)
    pool_out = nc.dram_tensor("pool_out", [G, C], F32)

    with nc.allow_low_precision(reason="bf16 table + tree-reduce validated within 2e-2 tolerance"), tile.TileContext(nc) as tc:
        with (
            tc.tile_pool(name="const", bufs=1) as cpool,
            tc.tile_pool(name="sbuf", bufs=2) as sb,
            tc.tile_pool(name="agg", bufs=1, space="PSUM") as ps_agg,
            tc.tile_pool(name="pxr", bufs=2, space="PSUM") as ps_xr,
            tc.tile_pool(name="ptf", bufs=1, space="PSUM") as ps_tf,
            tc.tile_pool(name="pmisc", bufs=1, space="PSUM") as ps_misc,
            tc.tile_pool(name="big", bufs=1) as mp,
        ):
            t_w1inv = cpool.tile([128, FW], BF16, name="t_w1inv")
            t_b1p = cpool.tile([128, FW], BF16, name="t_b1p")
            t_pma = cpool.tile([128, C], BF16, name="t_pma")
            t_pmb = cpool.tile([FW - 128, C], BF16, name="t_pmb")
            t_b2 = cpool.tile([128, 64], F32, name="t_b2")
            t_oh = cpool.tile([128, MTILES * G], BF16, name="t_oh")
            ident = cpool.tile([128, 128], BF16, name="ident")
            nc.sync.dma_start(out=t_w1inv[:], in_=w1inv_in[:])
            nc.sync.dma_start(out=t_b1p[:], in_=b1p_in[:])
            nc.sync.dma_start(out=t_pma[:], in_=pmat_in[0:128, :])
            nc.sync.dma_start(out=t_pmb[:], in_=pmat_in[128:FW, :])
            nc.sync.dma_start(out=t_b2[:], in_=b2_rep[:])
            nc.sync.dma_start(out=t_oh[:], in_=oh_in[:])
            make_identity(nc, ident[:])

            t_wl1 = cpool.tile([128, DPAD], BF16, name="t_wl1")
            t_wr1 = cpool.tile([128, DPAD], BF16, name="t_wr1")
            nc.sync.dma_start(out=t_wl1[:], in_=wl1[:])
            nc.sync.dma_start(out=t_wr1[:], in_=wr1[:])
            t_wl2a = cpool.tile([128, DPAD], BF16, name="t_wl2a")
            t_wl2b = cpool.tile([FW - 128, DPAD], BF16, name="t_wl2b")
            t_wr2a = cpool.tile([128, DPAD], BF16, name="t_wr2a")
            t_wr2b = cpool.tile([FW - 128, DPAD], BF16, name="t_wr2b")
            nc.sync.dma_start(out=t_wl2a[:], in_=wl2[0:128, :])
            nc.sync.dma_start(out=t_wl2b[:], in_=wl2[128:FW, :])
            nc.sync.dma_start(out=t_wr2a[:], in_=wr2[0:128, :])
            nc.sync.dma_start(out=t_wr2b[:], in_=wr2[128:FW, :])

            t_xT = mp.tile([128, NPAD], BF16, name="t_xT")
            nc.sync.dma_start(out=t_xT[:], in_=xT[:])
            h1T_a = mp.tile([128, NPAD], BF16, name="h1T_a")
            h1T_b = mp.tile([FW - 128, NPAD], BF16, name="h1T_b")

            def _rep_body(rep):
                R = f"_R{rep}"

                # ---------- layer transforms ----------
                def transform_tile(layer, t, which):
                    sl = slice(t * 128, (t + 1) * 128)
                    dtab = shard_tab if which == 0 else xr_tab
                    pst = ps_tf.tile([128, DPAD], F32, tag="tf",
                                     name=f"p{layer}_{t}_{which}{R}")
                    if layer == 1:
                        wt = t_wl1 if which == 0 else t_wr1
                        nc.tensor.matmul(pst[:], lhsT=t_xT[:, sl],
                                         rhs=wt[:], start=True, stop=True)
                    else:
                        wa = t_wl2a if which == 0 else t_wr2a
                        wb = t_wl2b if which == 0 else t_wr2b
                        nc.tensor.matmul(pst[:], lhsT=h1T_a[:, sl],
                                         rhs=wa[:], start=True, stop=False)
                        nc.tensor.matmul(pst[:], lhsT=h1T_b[:, sl],
                                         rhs=wb[:], start=False, stop=True)
                    stg = sb.tile([128, DPAD], BF16, tag="tfs",
                                  name=f"s{layer}_{t}_{which}{R}")
                    nc.scalar.copy(out=stg[:], in_=pst[:])
                    if which == 0:
                        nc.vector.memset(stg[:, FW:FW + 1], 1.0)
                    nc.sync.dma_start(out=dtab[sl, :], in_=stg[:])

                def transforms(layer):
                    # all xl tiles first so the AllGather starts before the
                    # local xr transform finishes
                    for which in (0, 1):
                        for t in range(MTILES):
                            transform_tile(layer, t, which)

                # ---------- edge layer ----------
                def edge_layer(layer):
                    for k in range(NSETS):
                        nch = set_nch[k]
                        (lo_a, lo_b), (hi_a, hi_b) = reg_off[k]
                        nsl = nch * 128
                        nlo = lo_b - lo_a
                        nhi = hi_b - hi_a
                        ti_xl = sb.tile([128, nsl // 16], I16, tag="ixl",
                                        name=f"ixl{layer}_{k}{R}")
                        nc.sync.dma_start(
                            out=ti_xl[:],
                            in_=xl_idx[:, lo_a // 16:lo_a // 16 + nsl // 16])
                        g_xl = sb.tile([128, nch, DPAD], BF16, tag="gxl",
                                       name=f"gxl{layer}_{k}{R}", bufs=3)
                        if nlo > 0:
                            nc.gpsimd.dma_gather(
                                out_ap=g_xl[:, 0:nlo // 128, :],
                                in_ap=glob_tab[0:NG:2, :],
                                idxs_ap=ti_xl[:, 0:nlo // 16],
                                num_idxs=nlo, num_idxs_reg=nlo, elem_size=DPAD,
                                elem_step=2 * DPAD,
                                single_packet=False, queue_num=0)
                        if nhi > 0:
                            nc.gpsimd.dma_gather(
                                out_ap=g_xl[:, nlo // 128:nch, :],
                                in_ap=glob_tab[1:NG:2, :],
                                idxs_ap=ti_xl[:, nlo // 16:nsl // 16],
                                num_idxs=nhi, num_idxs_reg=nhi, elem_size=DPAD,
                                elem_step=2 * DPAD,
                                single_packet=False, queue_num=1)
                        t_selT = sb.tile([BANK_NODES, nsl], BF16, tag="selt",
                                         name=f"selt{layer}_{k}{R}")
                        nc.sync.dma_start(out=t_selT[:],
                                          in_=selT_in[:, lo_a:lo_a + nsl])
                        t_xrb = sb.tile([BANK_NODES, NBANKS, DPAD], BF16,
                                        tag="xrb", name=f"xrb{layer}_{k}{R}")
                        nc.sync.dma_start(
                            out=t_xrb[:],
                            in_=xr_tab[k * SET_NODES:(k + 1) * SET_NODES, :]
                                .rearrange("(b s) d -> s b d", b=NBANKS))
                        # xr' broadcast: chunk-pair matmuls into one PSUM bank,
                        # single ACT copy per pair
                        t_xr = sb.tile([128, nch, DPAD], BF16, tag="txr",
                                       name=f"txr{layer}_{k}{R}")
                        for c2 in range(0, nch, 2):
                            n2 = min(2, nch - c2)
                            pxr = ps_xr.tile([128, 2, DPAD], F32, tag="xr",
                                             name=f"pxr{layer}_{k}_{c2}{R}")
                            for dc in range(n2):
                                c = c2 + dc
                                for (b, lo, hi) in chunk_segs[k][c]:
                                    nc.tensor.matmul(
                                        pxr[lo:hi, dc, :],
                                        lhsT=t_selT[:, c * 128 + lo:c * 128 + hi],
                                        rhs=t_xrb[:, b, :],
                                        start=True, stop=True)
                            nc.scalar.copy(out=t_xr[:, c2:c2 + n2, :],
                                           in_=pxr[:, 0:n2, :])
                        # u = xl' + xr'  (bf16, one 2x DVE pass)
                        t_u = sb.tile([128, nch, DPAD], BF16, tag="tu",
                                      name=f"tu{layer}_{k}{R}")
                        nc.vector.tensor_tensor(out=t_u[:], in0=g_xl[:],
                                                in1=t_xr[:],
                                                op=mybir.AluOpType.add)
                        # |u| over the 252 block cols: clear the bf16 sign bit
                        # (4x-mode tensor_scalar, in place; cols 252:256 --
                        # ones + al/ar -- stay signed)
                        tu_u16 = t_u[:, :, 0:FW].bitcast(mybir.dt.uint16)
                        nc.vector.tensor_scalar(
                            out=tu_u16, in0=tu_u16, scalar1=0x7FFF,
                            scalar2=None, op0=mybir.AluOpType.bitwise_and)
                        # signed tree: d = A - B then reduce 42 -> 1
                        va = t_u[:, :, 0:FW].rearrange("p c (h w) -> p c h w", h=H)
                        nc.vector.tensor_tensor(out=va[:, :, :, 0:42],
                                                in0=va[:, :, :, 0:42],
                                                in1=va[:, :, :, 42:84],
                                                op=mybir.AluOpType.subtract)
                        nc.vector.tensor_tensor(out=va[:, :, :, 0:21],
                                                in0=va[:, :, :, 0:21],
                                                in1=va[:, :, :, 21:42],
                                                op=mybir.AluOpType.add)
                        nc.vector.tensor_tensor(out=va[:, :, :, 0:5],
                                                in0=va[:, :, :, 0:5],
                                                in1=va[:, :, :, 16:21],
                                                op=mybir.AluOpType.add)
                        for w in (8, 4, 2, 1):
                            nc.vector.tensor_tensor(out=va[:, :, :, 0:w],
                                                    in0=va[:, :, :, 0:w],
                                                    in1=va[:, :, :, w:2 * w],
                                                    op=mybir.AluOpType.add)
                        # logit = (al+ar) + (A-B); exp
                        t_lg = sb.tile([128, nch, H], F32, tag="tlg",
                                       name=f"tlg{layer}_{k}{R}")
                        nc.vector.tensor_tensor(out=t_lg[:],
                                                in0=va[:, :, :, 0:1].squeeze(3),
                                                in1=t_u[:, :, DW:DW + H],
                                                op=mybir.AluOpType.add)
                        t_e = sb.tile([128, nch, H], BF16, tag="te",
                                      name=f"te{layer}_{k}{R}")
                        nc.scalar.activation(out=t_e[:], in_=t_lg[:],
                                             func=mybir.ActivationFunctionType.Exp)
                        t_sel = sb.tile([128, nch, 120], BF16, tag="tsel",
                                        name=f"tsel{layer}_{k}{R}")
                        ch0 = sum(set_nch[:k])
                        nc.sync.dma_start(out=t_sel[:],
                                          in_=sel_in[:, ch0 * 120:(ch0 + nch) * 120])
                        eb = bass.AP(t_e[:].tensor, t_e[:].offset,
                                     [list(t_e[:].ap[0]), [H, nch],
                                      [0, BANK_NODES], [1, H]])
                        nc.vector.tensor_tensor(
                            out=t_sel[:].rearrange("p c (s h) -> p c s h", h=H),
                            in0=t_sel[:].rearrange("p c (s h) -> p c s h", h=H),
                            in1=eb, op=mybir.AluOpType.mult)
                        pagg = ps_agg.tile([128, NBANKS * 512], F32, tag="pagg",
                                           name=f"pagg{layer}_{k}{R}")
                        for (col, b, lo, hi, st, sp_) in jobs[k]:
                            nc.tensor.matmul(
                                pagg[0:120, b * 512:b * 512 + DW],
                                lhsT=t_sel[lo:hi, col, :],
                                rhs=g_xl[lo:hi, col, 0:DW],
                                start=st, stop=sp_)
                        t_ev = sb.tile([128, NBANKS, DW], BF16, tag="tev",
                                       name=f"tev{layer}_{k}{R}")
                        pagg_v = bass.AP(pagg[:].tensor, pagg[:].offset,
                                         [list(pagg[:].ap[0]), [512, NBANKS], [1, DW]])
                        nc.scalar.copy(out=t_ev[:], in_=pagg_v)
                        t_dm = sb.tile([128, NBANKS], BF16, tag="tdm",
                                       name=f"tdm{layer}_{k}{R}")
                        nc.vector.tensor_scalar_max(
                            t_dm[:], t_ev[:, :, FW:FW + 1].squeeze(2), 1e-30)
                        t_d = sb.tile([128, NBANKS], F32, tag="td",
                                      name=f"td{layer}_{k}{R}")
                        nc.vector.reciprocal(out=t_d[:], in_=t_dm[:])
                        # normalize per bank (per-partition scalar, 4x)
                        for b in range(NBANKS):
                            nc.vector.tensor_scalar(
                                out=t_ev[:, b, 0:FW], in0=t_ev[:, b, 0:FW],
                                scalar1=t_d[:, b:b + 1], scalar2=None,
                                op0=mybir.AluOpType.mult)
                        nc.sync.dma_start(out=h_slots[k, :, :],
                                          in_=t_ev[:, :, 0:FW])

                # ---------- slot-major -> node-major reshape ----------
                def reshape_nodes(dst_tab):
                    # slot row 3*s + hh of (set k, bank b) -> node k*160+b*40+s
                    hvv = h_slots[:].rearrange("k p (b d) -> k p b d", b=NBANKS)
                    dv = dst_tab[:].rearrange("(k b s) d -> k b s d",
                                              k=NSETS, b=NBANKS)
                    for hh in range(H):
                        src = hvv[:, hh:hh + H * BANK_NODES:H, :, :]
                        for b in range(NBANKS):
                            nc.sync.dma_start(
                                out=dv[:, b, :, BW * hh:BW * hh + BW],
                                in_=src[:, :, b, BW * hh:BW * hh + BW])

                # ---------- run ----------
                transforms(1)
                nc.gpsimd.collective_compute(
                    "AllGather", mybir.AluOpType.bypass,
                    replica_groups=[list(range(NCORES))],
                    ins=[shard_tab[:].opt()], outs=[glob_tab[:].opt()])
                edge_layer(1)
                reshape_nodes(h1_node)

                # node-major L1: recover (1/0.4att), +b1, relu, build h1T
                for t in range(MTILES):
                    sl = slice(t * 128, (t + 1) * 128)
                    t_h = sb.tile([128, HROW], BF16, tag="th", name=f"th{t}{R}")
                    nc.sync.dma_start(out=t_h[:], in_=h1_node[sl, :])
                    th2 = sb.tile([128, FW], BF16, tag="th2", name=f"th2{t}{R}")
                    nc.vector.tensor_tensor(out=th2[:], in0=t_h[:, 0:FW],
                                            in1=t_w1inv[:],
                                            op=mybir.AluOpType.mult)
                    nc.vector.tensor_tensor(out=th2[:], in0=th2[:],
                                            in1=t_b1p[:],
                                            op=mybir.AluOpType.add)
                    nc.vector.tensor_scalar_max(th2[:], th2[:], 0.0)
                    for h2 in range(2):
                        wdt = 128 if h2 == 0 else FW - 128
                        ptr = ps_misc.tile([128, 512], BF16, tag="pm",
                                           name=f"ptr{t}_{h2}{R}")
                        nc.tensor.transpose(out=ptr[0:wdt, 0:128],
                                            in_=th2[:, h2 * 128:h2 * 128 + wdt],
                                            identity=ident[:])
                        dst = h1T_a if h2 == 0 else h1T_b
                        nc.vector.tensor_copy(out=dst[0:wdt, sl],
                                              in_=ptr[0:wdt, 0:128])
                    transform_tile(2, t, 0)
                    transform_tile(2, t, 1)

                nc.gpsimd.collective_compute(
                    "AllGather", mybir.AluOpType.bypass,
                    replica_groups=[list(range(NCORES))],
                    ins=[shard_tab[:].opt()], outs=[glob_tab[:].opt()])
                edge_layer(2)
                reshape_nodes(o2_node)

                # ---------- pooling: Pmat folds recovery+unpermute+mean ----
                ppool = ps_misc.tile([128, 512], F32, tag="pm", name=f"ppool{R}")
                for t in range(MTILES):
                    sl = slice(t * 128, (t + 1) * 128)
                    t_o = sb.tile([128, HROW], BF16, tag="to", name=f"to{t}{R}")
                    nc.sync.dma_start(out=t_o[:], in_=o2_node[sl, :])
                    toT_a = sb.tile([128, 128], BF16, tag="tta", name=f"tta{t}{R}")
                    toT_b = sb.tile([FW - 128, 128], BF16, tag="ttb",
                                    name=f"ttb{t}{R}")
                    for h2 in range(2):
                        wdt = 128 if h2 == 0 else FW - 128
                        ptr = ps_tf.tile([128, 512], BF16, tag="tf",
                                         name=f"otr{t}_{h2}{R}")
                        nc.tensor.transpose(out=ptr[0:wdt, 0:128],
                                            in_=t_o[:, h2 * 128:h2 * 128 + wdt],
                                            identity=ident[:])
                        dst = toT_a if h2 == 0 else toT_b
                        nc.scalar.copy(out=dst[:], in_=ptr[0:wdt, 0:128])
                    pmm = ps_xr.tile([128, 2, DPAD], F32, tag="xr",
                                     name=f"pmm{t}{R}")
                    nc.tensor.matmul(pmm[:, 0, 0:C], lhsT=toT_a[:],
                                     rhs=t_pma[:], start=True, stop=False)
                    nc.tensor.matmul(pmm[:, 0, 0:C], lhsT=toT_b[:],
                                     rhs=t_pmb[:], start=False, stop=True)
                    t_m = sb.tile([128, 64], F32, tag="tm", name=f"tm{t}{R}")
                    nc.vector.tensor_tensor(out=t_m[:], in0=pmm[:, 0, 0:C],
                                            in1=t_b2[:],
                                            op=mybir.AluOpType.add)
                    t_r = sb.tile([128, 64], BF16, tag="tr", name=f"tr{t}{R}")
                    nc.vector.tensor_scalar_max(t_r[:], t_m[:], 0.0)
                    nc.tensor.matmul(ppool[0:G, 0:64],
                                     lhsT=t_oh[:, t * G:(t + 1) * G], rhs=t_r[:],
                                     start=(t == 0), stop=(t == MTILES - 1))
                t_pl = sb.tile([G, C], F32, tag="tpl", name=f"t_pl{R}")
                nc.vector.tensor_copy(out=t_pl[:], in_=ppool[0:G, 0:64])
                nc.sync.dma_start(out=pool_in[:], in_=t_pl[:])
                nc.gpsimd.collective_compute(
                    "AllReduce", mybir.AluOpType.add,
                    replica_groups=[list(range(NCORES))],
                    ins=[pool_in[:].opt()], outs=[pool_out[:].opt()])
                t_pool = sb.tile([G, C], F32, tag="tpool", name=f"t_pool{R}")
                nc.sync.dma_start(out=t_pool[:], in_=pool_out[:])
                t_cnt = sb.tile([G, 1], F32, tag="tcnt", name=f"t_cnt{R}")
                nc.sync.dma_start(out=t_cnt[:], in_=cntr[:])
                nc.vector.tensor_scalar(out=t_pool[:], in0=t_pool[:],
                                        scalar1=t_cnt[:], scalar2=None,
                                        op0=mybir.AluOpType.mult)
                idf = cpool.tile([128, 128], F32, name=f"idf{R}")
                make_identity(nc, idf[:])
                ppt = ps_misc.tile([128, 512], F32, tag="pm", name=f"ppt{R}")
                nc.tensor.transpose(out=ppt[0:C, 0:G], in_=t_pool[:],
                                    identity=idf[0:G, 0:G])
                t_poolT = sb.tile([C, G], F32, tag="poolT", name=f"t_poolT{R}")
                nc.vector.tensor_copy(out=t_poolT[:], in_=ppt[0:C, 0:G])
                t_wc = sb.tile([C, NCLS], F32, tag="twc", name=f"t_wc{R}")
                nc.sync.dma_start(out=t_wc[:], in_=wc_in[:])
                plog = ps_misc.tile([128, 512], F32, tag="pm", name=f"plog{R}")
                nc.tensor.matmul(plog[0:G, 0:NCLS], lhsT=t_poolT[:], rhs=t_wc[:],
                                 start=True, stop=True)
                t_bc = sb.tile([G, NCLS], F32, tag="tbc", name=f"t_bc{R}")
                nc.sync.dma_start(out=t_bc[:], in_=bc_rep[:])
                t_log = sb.tile([G, NCLS], F32, tag="tlog", name=f"t_log{R}")
                nc.vector.tensor_tensor(out=t_log[:], in0=plog[0:G, 0:NCLS],
                                        in1=t_bc[:], op=mybir.AluOpType.add)
                t_ex = sb.tile([G, NCLS], F32, tag="tex", name=f"t_ex{R}")
                nc.scalar.activation(out=t_ex[:], in_=t_log[:],
                                     func=mybir.ActivationFunctionType.Exp)
                t_sm = sb.tile([G, 1], F32, tag="tsm", name=f"t_sm{R}")
                nc.vector.tensor_reduce(out=t_sm[:], in_=t_ex[:],
                                        axis=mybir.AxisListType.X,
                                        op=mybir.AluOpType.add)
                t_rc = sb.tile([G, 1], F32, tag="trc", name=f"t_rc{R}")
                nc.vector.reciprocal(out=t_rc[:], in_=t_sm[:])
                t_out = sb.tile([G, NCLS], F32, tag="tout", name=f"t_out{R}")
                nc.vector.tensor_scalar(out=t_out[:], in0=t_ex[:],
                                        scalar1=t_rc[:], scalar2=None,
                                        op0=mybir.AluOpType.mult)
                nc.sync.dma_start(out=out_ext[:], in_=t_out[:])

            for _rep in range(reps):
                _rep_body(_rep)

    nc.compile()
    return nc


# ------------------------------------------------------------------
# host weight transforms (v3)
# ------------------------------------------------------------------

def _att_layout(att):
    """Per head: 84-col layout [42 pos-att cols | 42 neg-att cols], -1 = pad."""
    lay = []
    for h in range(H):
        a = att[h]
        pos = [c for c in range(C) if a[c] >= 0]
        neg = [c for c in range(C) if a[c] < 0]
        assert len(pos) <= 42 and len(neg) <= 42, (len(pos), len(neg))
        lay.append(pos + [-1] * (42 - len(pos)) + neg + [-1] * (42 - len(neg)))
    return lay


def _wmat_v3(Wf, att, lay):
    """[in, 192] weights -> [in, 256]: 0.4*att premult permuted cols +
    0.6*(W @ att_h) linear-part cols at 253:256. Col 252 stays 0 (ones)."""
    out = np.zeros((Wf.shape[0], DPAD), np.float64)
    for h in range(H):
        for j, c in enumerate(lay[h]):
            if c >= 0:
                out[:, BW * h + j] = 0.4 * att[h, c] * Wf[:, C * h + c]
        out[:, DW + h] = 0.6 * (Wf[:, C * h:C * h + C] @ att[h])
    return out


def _perm_rows(Wf, lay):
    """[192, n] -> [252, n]: row 84h+j = orig row 64h+lay[h][j] (0 for pads)."""
    out = np.zeros((FW, Wf.shape[1]), np.float64)
    for h in range(H):
        for j, c in enumerate(lay[h]):
            if c >= 0:
                out[BW * h + j] = Wf[C * h + c]
    return out


def _vec_perm(vec_fn, lay):
    out = np.zeros(FW, np.float64)
    for h in range(H):
        for j, c in enumerate(lay[h]):
            if c >= 0:
                out[BW * h + j] = vec_fn(h, c)
    return out


# ------------------------------------------------------------------
# entry point
# ------------------------------------------------------------------

def prepare(x, edge_index, batch, Wl1, Wr1, att1, b1, Wl2, Wr2, att2, b2, Wc, bc,
            _reps=1):
    bf = ml_dtypes.bfloat16
    x = np.asarray(x, np.float32)
    cores, meta = _preprocess(edge_index, batch)
    core_of, pos_of = meta["core_of"], meta["pos_of"]

    att1f = np.asarray(att1, np.float64)
    att2f = np.asarray(att2, np.float64)
    b1f = np.asarray(b1, np.float64)
    lay1 = _att_layout(att1f)
    lay2 = _att_layout(att2f)

    wl1_v = _wmat_v3(np.asarray(Wl1, np.float64), att1f, lay1)
    wr1_v = _wmat_v3(np.asarray(Wr1, np.float64), att1f, lay1)
    wl2_v = _wmat_v3(_perm_rows(np.asarray(Wl2, np.float64), lay1), att2f, lay2)
    wr2_v = _wmat_v3(_perm_rows(np.asarray(Wr2, np.float64), lay1), att2f, lay2)
    w1inv = _vec_perm(lambda h, c: 1.0 / (0.4 * att1f[h, c]), lay1)
    b1p = _vec_perm(lambda h, c: b1f[C * h + c], lay1)
    pmat = np.zeros((FW, C), np.float64)
    for h in range(H):
        for j, c in enumerate(lay2[h]):
            if c >= 0:
                pmat[BW * h + j, c] = 1.0 / (3 * 0.4 * att2f[h, c])

    cnt = np.bincount(np.asarray(batch, np.int64), minlength=G).astype(np.float32)
    common = dict(
        wl1=np.ascontiguousarray(wl1_v.astype(bf)),
        wr1=np.ascontiguousarray(wr1_v.astype(bf)),
        wl2=np.ascontiguousarray(wl2_v.astype(bf)),
        wr2=np.ascontiguousarray(wr2_v.astype(bf)),
        w1inv_rep=np.ascontiguousarray(
            np.tile(w1inv.reshape(1, FW), (128, 1)).astype(bf)),
        b1p_rep=np.ascontiguousarray(
            np.tile(b1p.reshape(1, FW), (128, 1)).astype(bf)),
        pmat=np.ascontiguousarray(pmat.astype(bf)),
        b2_rep=np.ascontiguousarray(
            np.tile(np.asarray(b2, np.float32).reshape(1, 64), (128, 1))),
        wc=np.ascontiguousarray(np.asarray(Wc, np.float32)),
        bc_rep=np.ascontiguousarray(
            np.tile(np.asarray(bc, np.float32).reshape(1, NCLS), (G, 1))),
        cnt_recip=np.ascontiguousarray(
            (1.0 / np.maximum(cnt, 1.0)).reshape(G, 1)),
    )

    nc = _build(meta, reps=_reps)

    in_maps = []
    for c in range(NCORES):
        im = dict(common)
        xpad = np.zeros((NPAD, F), np.float32)
        mine = np.nonzero(core_of == c)[0]
        xpad[pos_of[mine]] = x[mine]
        im["xT"] = np.ascontiguousarray(xpad.T.astype(bf))
        im["oh"] = _onehots(batch, c, core_of, pos_of)
        im["xl_idx16"] = cores[c]["xl_idx16"]
        im["sel01"] = cores[c]["sel01"]
        im["selT"] = cores[c]["selT"]
        in_maps.append(im)

    return nc, in_maps


def kernel(x, edge_index, batch, Wl1, Wr1, att1, b1, Wl2, Wr2, att2, b2, Wc, bc,
           _want_trace=False):
    nc, in_maps = prepare(x, edge_index, batch, Wl1, Wr1, att1, b1,
                          Wl2, Wr2, att2, b2, Wc, bc)
    res = bass_utils.run_bass_kernel_spmd(
        nc, in_maps, core_ids=list(range(NCORES)), trace=_want_trace)
    out = np.asarray(res.results[0]["out"], np.float32)
    kernel._last_exec_ns = getattr(res, "exec_time_ns", None)
    return out

